# revision 18
# baseline (speedup 1.0000x reference)
"""Trainium2 Bass kernel for the 2-layer CIN — v3 (host-z0 + square trick).

Reference computation (per batch b, channel d):
  z0[hf]  = x[h,d,b] * x[f,d,b]            (h,f in 0..39)
  x1[o]   = relu(sum_hf W0[o,hf,d] z0[hf] + b0[o])
  z1[hf]  = x1[h,d,b] * x[f,d,b]           (h in 0..63)
  x2[o]   = relu(sum_hf W1[o,hf,d] z1[hf] + b1[o])
  out[b]  = [sum_d x | sum_d x1 | sum_d x2]            (2048, 168)

Sharding: pure 8-way split of the embedding dim D=32 -> DC=4 channels per
core, full batch per core; host adds the 8 partial d-sums. Makes per-core
weight traffic 8x smaller than a batch split.

Device algorithm (bf16 compute, fp32 PSUM):
  Column space: 8192 = 2 d-pairs x 4 b-chunks x (d_even 512 | d_odd 512).
  * Layer 1: z0 = x (x) x is static, so the HOST precomputes the 820
    symmetric-folded products and the device just streams z0 tiles from
    DRAM into the accumulating matmuls. No on-device build at all.
  * Layer 2 A-tiles (square trick): a 128-col "sel2" stationary with two
    ones per column computes u[p] = x1[h(p)] + x[f(p)] on the PE into PSUM;
    ScalarE Square evacuates z = (u/sqrt2)^2 = u^2/2, exploiting
    x1_h*x_f = (x1_h+x_f)^2/2 - x1_h^2/2 - x_f^2/2; the s-corrections fold
    into one correction matmul per d against s = [x^2 | x1^2].
  * Layer 2 C-tiles: XH = x1[3t+p//40] via SBUF->SBUF partition-replicating
    DMA; z = XH * XFR on VectorE (2x mode) with XFR[p] = x[p%40] resident.
  Real matmuls: per tile, two 64-col stationaries (one per d of the pair)
  accumulate into disjoint PSUM partition halves. ScalarE applies bias+relu
  into the joint source XJ (x rows 0-39, x1 rows 64-127); VectorE
  accumulates d-sums into a (128, B) fp32 acc (acc2 rows 0-63, acc1 rows
  64-127); a PE-transpose epilogue emits (2048, 128) fp32 per core.
"""

import os
from contextlib import ExitStack

import numpy as np
import ml_dtypes

import concourse.bass as bass
import concourse.bacc as bacc
import concourse.tile as tile
from concourse import mybir
from concourse.bass_utils import run_bass_kernel_spmd
from concourse.masks import make_identity

BF16 = mybir.dt.bfloat16
FP32 = mybir.dt.float32
NPBF16 = ml_dtypes.bfloat16

B, F, D = 2048, 40, 32
O0, O1 = 64, 64
NCORES = 8
DC = D // NCORES            # 4 embedding channels per core
NPAIR = DC // 2             # 2 d-pairs
NCHUNK = 4                  # batch chunks of 512
BC = B // NCHUNK            # 512 batch cols per chunk
NG = NPAIR * NCHUNK         # 8 col groups of 1024
PITCH = DC * B              # 8192 free cols of the resident tiles
JX1 = 64                    # x1 rows start at partition 64 of XJ

# --- tunables ---
HC2 = 33                    # layer-2 h-values covered by C-tiles (mult of 3)
NC2 = HC2 * F // 120        # 8 C-tiles per group

L1_PAIRS = [(h, f) for h in range(F) for f in range(h, F)]          # 820
L2_PAIRS = [(h, f) for h in range(HC2, O0) for f in range(F)]       # 1600
ND1 = (len(L1_PAIRS) + 127) // 128                                  # 7
NA2 = (len(L2_PAIRS) + 127) // 128                                  # 13


def _build_bass(reps=1):
    nc = bacc.Bacc()
    xt = nc.declare_dram_parameter("xt", [F, PITCH], BF16, isOutput=False)
    z0 = nc.declare_dram_parameter("z0", [128, ND1 * PITCH], BF16, isOutput=False)
    w1t = nc.declare_dram_parameter("w1t", [128, ND1 * DC * O0], BF16, isOutput=False)
    wa = nc.declare_dram_parameter("wa", [128, NA2 * DC * O0], BF16, isOutput=False)
    wc = nc.declare_dram_parameter("wc", [120, NC2 * DC * O1], BF16, isOutput=False)
    sel2 = nc.declare_dram_parameter("sel2", [128, NA2 * 128], BF16, isOutput=False)
    cor2 = nc.declare_dram_parameter("cor2", [128, DC * O1], BF16, isOutput=False)
    b0 = nc.declare_dram_parameter("b0", [O0, 1], FP32, isOutput=False)
    b1 = nc.declare_dram_parameter("b1", [O1, 1], FP32, isOutput=False)
    out = nc.declare_dram_parameter("out", [B, O0 + O1], FP32, isOutput=True)

    with ExitStack() as ctx:
        tc = ctx.enter_context(tile.TileContext(nc))
        singles = ctx.enter_context(tc.tile_pool(name="singles", bufs=1))
        u_ps = ctx.enter_context(tc.tile_pool(name="u_ps", bufs=2, space="PSUM"))
        y_ps = ctx.enter_context(tc.tile_pool(name="y_ps", bufs=4, space="PSUM"))
        z_sb = ctx.enter_context(tc.tile_pool(name="z_sb", bufs=10))
        xh_sb = ctx.enter_context(tc.tile_pool(name="xh_sb", bufs=12))
        x2_sb = ctx.enter_context(tc.tile_pool(name="x2_sb", bufs=4))
        o_sb = ctx.enter_context(tc.tile_pool(name="o_sb", bufs=2))
        s_sb = ctx.enter_context(tc.tile_pool(name="s_sb", bufs=8))
        z0_sb = ctx.enter_context(tc.tile_pool(name="z0_sb", bufs=8))

        # ---- resident tensors ----
        xj = singles.tile([128, PITCH], BF16)   # x rows 0-39, x1 rows 64-127
        xfr = singles.tile([128, PITCH], BF16)  # x[p%40], pad 120-127
        w1s = singles.tile([128, ND1, DC * O0], BF16)
        was = singles.tile([128, NA2, DC * O0], BF16)
        wcs = singles.tile([120, NC2, DC * O1], BF16)
        sel2s = singles.tile([128, NA2, 128], BF16)
        cor2s = singles.tile([128, DC * O1], BF16)
        b0s = singles.tile([O0, 1], FP32)
        b1s = singles.tile([O1, 1], FP32)
        acc12 = singles.tile([128, B], FP32)    # rows 0-63 acc2, 64-127 acc1
        ident = singles.tile([128, 128], FP32)
        make_identity(nc, ident)

        xt_ap = xt[:]
        rep_src = bass.AP(
            tensor=xt_ap.tensor, offset=xt_ap.offset,
            ap=[[0, 3], [PITCH, F], [1, PITCH]],
        )
        pad_src = bass.AP(
            tensor=xt_ap.tensor, offset=xt_ap.offset,
            ap=[[PITCH, 8], [1, PITCH]],
        )
        pad24_src = bass.AP(
            tensor=xt_ap.tensor, offset=xt_ap.offset,
            ap=[[PITCH, JX1 - F], [1, PITCH]],
        )

        def load_inputs():
            nc.gpsimd.dma_start(out=xj[0:F, :], in_=xt[:])
            nc.gpsimd.dma_start(out=xj[F:JX1, :], in_=pad24_src)
            nc.gpsimd.dma_start(out=xfr[0:3 * F, :], in_=rep_src)
            nc.gpsimd.dma_start(out=xfr[3 * F:128, :], in_=pad_src)
            nc.gpsimd.dma_start(out=sel2s, in_=sel2[:])
            nc.gpsimd.dma_start(out=cor2s, in_=cor2[:])
            nc.gpsimd.dma_start(out=b0s, in_=b0[:])
            nc.gpsimd.dma_start(out=b1s, in_=b1[:])
            nc.sync.dma_start(out=w1s, in_=w1t[:])
            nc.sync.dma_start(out=was, in_=wa[:])
            nc.gpsimd.dma_start(out=wcs, in_=wc[:])

        xj_ap = xj[:]

        load_inputs()
        for rep in range(reps):
            nc.vector.memset(acc12, 0.0)
            for pair in range(NPAIR):
                dbase = pair * 2
                # stream this pair's z0 tiles (prefetched via pool bufs)
                z0ts = []
                for t in range(ND1):
                    zt = z0_sb.tile([128, NCHUNK * 1024], BF16, tag="z0")
                    nc.gpsimd.dma_start(
                        out=zt,
                        in_=z0[:, t * PITCH + pair * NCHUNK * 1024:
                               t * PITCH + (pair + 1) * NCHUNK * 1024],
                    )
                    z0ts.append(zt)

                for chunk in range(NCHUNK):
                    g = pair * NCHUNK + chunk
                    col0 = g * 1024
                    ccol = chunk * 1024
                    bcol = chunk * 512
                    s = s_sb.tile([128, 1024], BF16, tag="s")

                    # ---- layer 1: stream host-built z0 into matmuls ----
                    y0 = y_ps.tile([128, 512], FP32, tag="y", name=f"y0_{g}")
                    for t in range(ND1):
                        for i in range(2):
                            nc.tensor.matmul(
                                y0[i * 64:(i + 1) * 64, :],
                                lhsT=w1s[:, t, (dbase + i) * 64:
                                         (dbase + i + 1) * 64],
                                rhs=z0ts[t][:, ccol + i * 512:
                                            ccol + (i + 1) * 512],
                                start=(t == 0), stop=(t == ND1 - 1),
                                skip_group_check=True,
                            )
                    for i in range(2):
                        nc.scalar.activation(
                            out=xj[JX1:JX1 + O0,
                                   col0 + i * 512: col0 + (i + 1) * 512],
                            in_=y0[i * 64:(i + 1) * 64, :],
                            func=mybir.ActivationFunctionType.Relu,
                            bias=b0s, scale=1.0,
                        )
                        nc.vector.tensor_add(
                            acc12[64:128, bcol:bcol + 512],
                            acc12[64:128, bcol:bcol + 512],
                            xj_ap[JX1:JX1 + O0,
                                  col0 + i * 512:col0 + (i + 1) * 512],
                        )

                    # ---- layer 2 ----
                    y1 = y_ps.tile([128, 512], FP32, tag="y", name=f"y1_{g}")
                    # C-tile replication DMAs first (overlap A compute)
                    xhs = []
                    for t in range(NC2):
                        xh = xh_sb.tile([120, 1024], BF16, tag="xh")
                        srcap = bass.AP(
                            tensor=xj_ap.tensor,
                            offset=(JX1 + 3 * t) * PITCH + col0,
                            ap=[[PITCH, 3], [0, F], [1, 1024]],
                        )
                        # split replication DMAs across both HWDGE rings
                        eng = nc.scalar if t % 2 == 1 else nc.sync
                        eng.dma_start(out=xh, in_=srcap)
                        xhs.append(xh)
                    # A-tiles (square trick, ScalarE evac)
                    for t in range(NA2):
                        u = u_ps.tile([128, 1024], FP32, tag="u")
                        for hh in range(2):
                            nc.tensor.matmul(
                                u[:, hh * 512:(hh + 1) * 512],
                                lhsT=sel2s[:, t, :],
                                rhs=xj_ap[:, col0 + hh * 512:
                                          col0 + (hh + 1) * 512],
                                start=True, stop=True,
                            )
                        z = z_sb.tile([128, 1024], BF16, tag="z")
                        nc.scalar.activation(
                            out=z, in_=u,
                            func=mybir.ActivationFunctionType.Square,
                            bias=0.0, scale=float(1.0 / np.sqrt(2.0)),
                        )
                        for i in range(2):
                            nc.tensor.matmul(
                                y1[i * 64:(i + 1) * 64, :],
                                lhsT=was[:, t, (dbase + i) * 64:
                                         (dbase + i + 1) * 64],
                                rhs=z[:, i * 512:(i + 1) * 512],
                                start=(t == 0), stop=False,
                                skip_group_check=True,
                            )
                    # C-tiles: VectorE mul + matmuls
                    for t in range(NC2):
                        z = z_sb.tile([120, 1024], BF16, tag="z")
                        nc.vector.tensor_mul(
                            z, xhs[t], xfr[0:120, col0:col0 + 1024]
                        )
                        for i in range(2):
                            nc.tensor.matmul(
                                y1[i * 64:(i + 1) * 64, :],
                                lhsT=wcs[:, t, (dbase + i) * 64:
                                         (dbase + i + 1) * 64],
                                rhs=z[:, i * 512:(i + 1) * 512],
                                start=False, stop=False,
                                skip_group_check=True,
                            )
                    # correction: s = [x^2 | garbage | x1^2], one mm per d
                    nc.scalar.activation(
                        out=s[0:JX1, :], in_=xj_ap[0:JX1, col0:col0 + 1024],
                        func=mybir.ActivationFunctionType.Square,
                        bias=0.0, scale=1.0,
                    )
                    nc.scalar.activation(
                        out=s[JX1:128, :],
                        in_=xj_ap[JX1:128, col0:col0 + 1024],
                        func=mybir.ActivationFunctionType.Square,
                        bias=0.0, scale=1.0,
                    )
                    for i in range(2):
                        nc.tensor.matmul(
                            y1[i * 64:(i + 1) * 64, :],
                            lhsT=cor2s[:, (dbase + i) * 64:
                                       (dbase + i + 1) * 64],
                            rhs=s[:, i * 512:(i + 1) * 512],
                            start=False, stop=True,
                            skip_group_check=True,
                        )
                    for i in range(2):
                        x2 = x2_sb.tile([O1, 512], BF16, tag="x2")
                        nc.scalar.activation(
                            out=x2, in_=y1[i * 64:(i + 1) * 64, :],
                            func=mybir.ActivationFunctionType.Relu,
                            bias=b1s, scale=1.0,
                        )
                        nc.vector.tensor_add(
                            acc12[0:64, bcol:bcol + 512],
                            acc12[0:64, bcol:bcol + 512], x2
                        )

            # ---- epilogue: transpose (128, B) acc -> (B, 128) fp32 ----
            for bh in range(B // 128):
                outT = o_sb.tile([128, O0 + O1], FP32, tag="outT")
                pt = y_ps.tile([128, 128], FP32, tag="y")
                nc.tensor.transpose(
                    pt, acc12[:, bh * 128:(bh + 1) * 128], ident
                )
                nc.vector.tensor_copy(out=outT[:, 0:64], in_=pt[:, 64:128])
                nc.vector.tensor_copy(out=outT[:, 64:128], in_=pt[:, 0:64])
                nc.sync.dma_start(
                    out=out[bh * 128:(bh + 1) * 128, :], in_=outT
                )

    nc.compile()
    return nc


_NC_CACHE = {}
LAST_RESULT = None


def _get_nc(reps=1):
    if reps not in _NC_CACHE:
        _NC_CACHE[reps] = _build_bass(reps)
    return _NC_CACHE[reps]


def _host_prep(x, W0, b0, W1, b1):
    """Per-core input maps. Core c handles d-channels [4c, 4c+4)."""
    x = np.asarray(x, dtype=np.float32)
    W0 = np.asarray(W0, dtype=np.float32)
    W1 = np.asarray(W1, dtype=np.float32)

    # fold W0 to upper-triangular pairs: (o, 820, d)
    W0r = W0.reshape(O0, F, F, D)
    iu0, iu1 = np.triu_indices(F)
    W0f = W0r[:, iu0, iu1, :] + np.where(
        (iu0 != iu1)[None, :, None], W0r[:, iu1, iu0, :], 0.0
    )
    W1r = W1.reshape(O1, O0, F, D)

    in_maps = []
    for c in range(NCORES):
        ds = slice(4 * c, 4 * c + 4)
        xc = x[:, :, ds]                           # (2048, 40, 4)

        # col layout: pair p, chunk k, d_par i, b_off
        xt = np.empty((F, NPAIR, NCHUNK, 2, BC), dtype=np.float32)
        for p in range(NPAIR):
            for i in range(2):
                d = 2 * p + i
                xt[:, p, :, i, :] = xc[:, :, d].T.reshape(F, NCHUNK, BC)
        xtc = np.ascontiguousarray(xt.reshape(F, PITCH)).astype(NPBF16)

        # host z0: (128, ND1 * PITCH); tile t rows = folded pairs
        z0h = np.zeros((128, ND1, NPAIR, NCHUNK, 2, BC), dtype=NPBF16)
        prod = xc[:, iu0, :] * xc[:, iu1, :]       # (B, 820, 4) fp32
        for t in range(ND1):
            n = min(128, 820 - t * 128)
            blk = prod[:, t * 128:t * 128 + n, :]  # (B, n, 4)
            # -> (n, d, B) -> (n, pair, i, chunk, b) -> (n, pair, chunk, i, b)
            pb = blk.transpose(1, 2, 0).reshape(n, NPAIR, 2, NCHUNK, BC)
            z0h[:n, t] = pb.transpose(0, 1, 3, 2, 4).astype(NPBF16)
        z0hc = np.ascontiguousarray(z0h.reshape(128, ND1 * PITCH))

        # layer-1 weights (128, ND1, DC*64) ordered [tile][d][o]
        w1w = np.zeros((128, ND1, DC, O0), dtype=np.float32)
        w0fc = W0f[:, :, ds]                       # (64, 820, 4)
        for t in range(ND1):
            n = min(128, 820 - t * 128)
            w1w[:n, t] = w0fc[:, t * 128:t * 128 + n, :].transpose(1, 2, 0)
        w1tc = np.ascontiguousarray(
            w1w.reshape(128, ND1 * DC * O0)).astype(NPBF16)

        # layer-2 A-tile weights
        w1c = W1r[:, :, :, ds]                     # (64, 64, 40, 4)
        wa = np.zeros((128, NA2, DC, O0), dtype=np.float32)
        for t in range(NA2):
            rows = L2_PAIRS[t * 128:(t + 1) * 128]
            for r, (h, f) in enumerate(rows):
                wa[r, t] = w1c[:, h, f, :].T
        wac = np.ascontiguousarray(
            wa.reshape(128, NA2 * DC * O0)).astype(NPBF16)

        # C-tile weights (120, NC2, DC*64): rows h=3t+p//40, f=p%40
        wcw = np.zeros((120, NC2, DC, O1), dtype=np.float32)
        for t in range(NC2):
            for p in range(120):
                h, f = 3 * t + p // F, p % F
                wcw[p, t] = w1c[:, h, f, :].T
        wcc = np.ascontiguousarray(
            wcw.reshape(120, NC2 * DC * O1)).astype(NPBF16)

        # layer-2 correction
        cor2 = np.zeros((128, DC, O1), dtype=np.float32)
        for (h, f) in L2_PAIRS:
            w = w1c[:, h, f, :]                    # (o, d)
            cor2[f] -= 0.5 * w.T
            cor2[64 + h] -= 0.5 * w.T

        in_maps.append({
            "xt": xtc,
            "z0": z0hc,
            "w1t": w1tc,
            "wa": wac,
            "wc": wcc,
            "sel2": _SEL2,
            "cor2": np.ascontiguousarray(
                cor2.reshape(128, DC * O1)).astype(NPBF16),
            "b0": np.asarray(b0, np.float32).reshape(O0, 1),
            "b1": np.asarray(b1, np.float32).reshape(O1, 1),
        })
    return in_maps


def _build_sels():
    s2 = np.zeros((128, NA2, 128), dtype=NPBF16)
    for t in range(NA2):
        rows = L2_PAIRS[t * 128:(t + 1) * 128]
        for p, (h, f) in enumerate(rows):
            s2[f, t, p] += 1.0
            s2[64 + h, t, p] += 1.0
    return np.ascontiguousarray(s2.reshape(128, NA2 * 128))


_SEL2 = _build_sels()


def kernel(x, W0, b0, W1, b1):
    global LAST_RESULT
    x = np.asarray(x, dtype=np.float32)

    nc = _get_nc()
    in_maps = _host_prep(x, W0, b0, W1, b1)
    res = run_bass_kernel_spmd(nc, in_maps, core_ids=list(range(NCORES)))
    LAST_RESULT = res

    out = np.empty((B, F + O0 + O1), dtype=np.float32)
    out[:, :F] = x.sum(axis=-1)
    acc = np.zeros((B, O0 + O1), dtype=np.float32)
    for c in range(NCORES):
        acc += np.asarray(res.results[c]["out"])
    out[:, F:] = acc
    return out


# revision 19
# speedup vs baseline: 1.0934x; 1.0934x over previous
"""Trainium2 Bass kernel for the 2-layer CIN — v3 (host-z0 + square trick).

Reference computation (per batch b, channel d):
  z0[hf]  = x[h,d,b] * x[f,d,b]            (h,f in 0..39)
  x1[o]   = relu(sum_hf W0[o,hf,d] z0[hf] + b0[o])
  z1[hf]  = x1[h,d,b] * x[f,d,b]           (h in 0..63)
  x2[o]   = relu(sum_hf W1[o,hf,d] z1[hf] + b1[o])
  out[b]  = [sum_d x | sum_d x1 | sum_d x2]            (2048, 168)

Sharding: pure 8-way split of the embedding dim D=32 -> DC=4 channels per
core, full batch per core; host adds the 8 partial d-sums. Makes per-core
weight traffic 8x smaller than a batch split.

Device algorithm (bf16 compute, fp32 PSUM):
  Column space: 8192 = 2 d-pairs x 4 b-chunks x (d_even 512 | d_odd 512).
  * Layer 1: z0 = x (x) x is static, so the HOST precomputes the 820
    symmetric-folded products and the device just streams z0 tiles from
    DRAM into the accumulating matmuls. No on-device build at all.
  * Layer 2 A-tiles (square trick): a 128-col "sel2" stationary with two
    ones per column computes u[p] = x1[h(p)] + x[f(p)] on the PE into PSUM;
    ScalarE Square evacuates z = (u/sqrt2)^2 = u^2/2, exploiting
    x1_h*x_f = (x1_h+x_f)^2/2 - x1_h^2/2 - x_f^2/2; the s-corrections fold
    into one correction matmul per d against s = [x^2 | x1^2].
  * Layer 2 C-tiles: XH = x1[3t+p//40] via SBUF->SBUF partition-replicating
    DMA; z = XH * XFR on VectorE (2x mode) with XFR[p] = x[p%40] resident.
  Real matmuls: per tile, two 64-col stationaries (one per d of the pair)
  accumulate into disjoint PSUM partition halves. ScalarE applies bias+relu
  into the joint source XJ (x rows 0-39, x1 rows 64-127); VectorE
  accumulates d-sums into a (128, B) fp32 acc (acc2 rows 0-63, acc1 rows
  64-127); a PE-transpose epilogue emits (2048, 128) fp32 per core.
"""

import os
from contextlib import ExitStack

import numpy as np
import ml_dtypes

import concourse.bass as bass
import concourse.bacc as bacc
import concourse.tile as tile
from concourse import mybir
from concourse.bass_utils import run_bass_kernel_spmd
from concourse.masks import make_identity

BF16 = mybir.dt.bfloat16
FP32 = mybir.dt.float32
NPBF16 = ml_dtypes.bfloat16

B, F, D = 2048, 40, 32
O0, O1 = 64, 64
NCORES = 8
DC = D // NCORES            # 4 embedding channels per core
NPAIR = DC // 2             # 2 d-pairs
NCHUNK = 4                  # batch chunks of 512
BC = B // NCHUNK            # 512 batch cols per chunk
NG = NPAIR * NCHUNK         # 8 col groups of 1024
PITCH = DC * B              # 8192 free cols of the resident tiles
JX1 = 64                    # x1 rows start at partition 64 of XJ

# --- tunables ---
HC2 = 33                    # layer-2 h-values covered by C-tiles (mult of 3)
NC2 = HC2 * F // 120        # 8 C-tiles per group

L1_PAIRS = [(h, f) for h in range(F) for f in range(h, F)]          # 820
L2_PAIRS = [(h, f) for h in range(HC2, O0) for f in range(F)]       # 1600
ND1 = (len(L1_PAIRS) + 127) // 128                                  # 7
NA2 = (len(L2_PAIRS) + 127) // 128                                  # 13


def _build_bass(reps=1):
    nc = bacc.Bacc()
    xt = nc.declare_dram_parameter("xt", [F, PITCH], BF16, isOutput=False)
    z0 = nc.declare_dram_parameter("z0", [128, ND1 * PITCH], BF16, isOutput=False)
    w1t = nc.declare_dram_parameter("w1t", [128, ND1 * DC * O0], BF16, isOutput=False)
    wa = nc.declare_dram_parameter("wa", [128, NA2 * DC * O0], BF16, isOutput=False)
    wc = nc.declare_dram_parameter("wc", [120, NC2 * DC * O1], BF16, isOutput=False)
    sel2 = nc.declare_dram_parameter("sel2", [128, NA2 * 128], BF16, isOutput=False)
    cor2 = nc.declare_dram_parameter("cor2", [128, DC * O1], BF16, isOutput=False)
    b0 = nc.declare_dram_parameter("b0", [O0, 1], FP32, isOutput=False)
    b1 = nc.declare_dram_parameter("b1", [O1, 1], FP32, isOutput=False)
    out = nc.declare_dram_parameter("out", [B, O0 + O1], FP32, isOutput=True)

    with ExitStack() as ctx:
        tc = ctx.enter_context(tile.TileContext(nc))
        singles = ctx.enter_context(tc.tile_pool(name="singles", bufs=1))
        u_ps = ctx.enter_context(tc.tile_pool(name="u_ps", bufs=2, space="PSUM"))
        y_ps = ctx.enter_context(tc.tile_pool(name="y_ps", bufs=4, space="PSUM"))
        z_sb = ctx.enter_context(tc.tile_pool(name="z_sb", bufs=10))
        xh_sb = ctx.enter_context(tc.tile_pool(name="xh_sb", bufs=12))
        x2_sb = ctx.enter_context(tc.tile_pool(name="x2_sb", bufs=4))
        o_sb = ctx.enter_context(tc.tile_pool(name="o_sb", bufs=2))
        s_sb = ctx.enter_context(tc.tile_pool(name="s_sb", bufs=8))
        z0_sb = ctx.enter_context(tc.tile_pool(name="z0_sb", bufs=8))

        # ---- resident tensors ----
        xj = singles.tile([128, PITCH], BF16)   # x rows 0-39, x1 rows 64-127
        xfr = singles.tile([128, PITCH], BF16)  # x[p%40], pad 120-127
        w1s = singles.tile([128, ND1, DC * O0], BF16)
        was = singles.tile([128, NA2, DC * O0], BF16)
        wcs = singles.tile([120, NC2, DC * O1], BF16)
        sel2s = singles.tile([128, NA2, 128], BF16)
        cor2s = singles.tile([128, DC * O1], BF16)
        b0s = singles.tile([O0, 1], FP32)
        b1s = singles.tile([O1, 1], FP32)
        acc12 = singles.tile([128, B], FP32)    # rows 0-63 acc2, 64-127 acc1
        ident = singles.tile([128, 128], FP32)
        make_identity(nc, ident)

        xt_ap = xt[:]
        rep_src = bass.AP(
            tensor=xt_ap.tensor, offset=xt_ap.offset,
            ap=[[0, 3], [PITCH, F], [1, PITCH]],
        )
        pad_src = bass.AP(
            tensor=xt_ap.tensor, offset=xt_ap.offset,
            ap=[[PITCH, 8], [1, PITCH]],
        )
        pad24_src = bass.AP(
            tensor=xt_ap.tensor, offset=xt_ap.offset,
            ap=[[PITCH, JX1 - F], [1, PITCH]],
        )

        def load_inputs():
            nc.gpsimd.dma_start(out=xj[0:F, :], in_=xt[:])
            nc.gpsimd.dma_start(out=xj[F:JX1, :], in_=pad24_src)
            nc.gpsimd.dma_start(out=xfr[0:3 * F, :], in_=rep_src)
            nc.gpsimd.dma_start(out=xfr[3 * F:128, :], in_=pad_src)
            nc.gpsimd.dma_start(out=sel2s, in_=sel2[:])
            nc.gpsimd.dma_start(out=cor2s, in_=cor2[:])
            nc.gpsimd.dma_start(out=b0s, in_=b0[:])
            nc.gpsimd.dma_start(out=b1s, in_=b1[:])
            nc.sync.dma_start(out=w1s, in_=w1t[:])
            nc.sync.dma_start(out=was, in_=wa[:])
            nc.gpsimd.dma_start(out=wcs, in_=wc[:])

        xj_ap = xj[:]

        load_inputs()
        for rep in range(reps):
            nc.vector.memset(acc12, 0.0)
            for pair in range(NPAIR):
                dbase = pair * 2
                # stream this pair's z0 tiles (prefetched via pool bufs)
                z0ts = []
                for t in range(ND1):
                    zt = z0_sb.tile([128, NCHUNK * 1024], BF16, tag="z0")
                    nc.gpsimd.dma_start(
                        out=zt,
                        in_=z0[:, t * PITCH + pair * NCHUNK * 1024:
                               t * PITCH + (pair + 1) * NCHUNK * 1024],
                    )
                    z0ts.append(zt)

                for chunk in range(NCHUNK):
                    g = pair * NCHUNK + chunk
                    col0 = g * 1024
                    ccol = chunk * 1024
                    bcol = chunk * 512
                    s = s_sb.tile([128, 1024], BF16, tag="s")

                    # ---- layer 1: stream host-built z0 into matmuls ----
                    y0 = y_ps.tile([128, 512], FP32, tag="y", name=f"y0_{g}")
                    for t in range(ND1):
                        for i in range(2):
                            nc.tensor.matmul(
                                y0[i * 64:(i + 1) * 64, :],
                                lhsT=w1s[:, t, (dbase + i) * 64:
                                         (dbase + i + 1) * 64],
                                rhs=z0ts[t][:, ccol + i * 512:
                                            ccol + (i + 1) * 512],
                                start=(t == 0), stop=(t == ND1 - 1),
                                skip_group_check=True,
                            )
                    for i in range(2):
                        nc.scalar.activation(
                            out=xj[JX1:JX1 + O0,
                                   col0 + i * 512: col0 + (i + 1) * 512],
                            in_=y0[i * 64:(i + 1) * 64, :],
                            func=mybir.ActivationFunctionType.Relu,
                            bias=b0s, scale=1.0,
                        )
                        nc.vector.tensor_add(
                            acc12[64:128, bcol:bcol + 512],
                            acc12[64:128, bcol:bcol + 512],
                            xj_ap[JX1:JX1 + O0,
                                  col0 + i * 512:col0 + (i + 1) * 512],
                        )

                    # ---- layer 2 ----
                    y1 = y_ps.tile([128, 512], FP32, tag="y", name=f"y1_{g}")
                    # C-tile replication DMAs first (overlap A compute)
                    xhs = []
                    for t in range(NC2):
                        xh = xh_sb.tile([120, 1024], BF16, tag="xh")
                        srcap = bass.AP(
                            tensor=xj_ap.tensor,
                            offset=(JX1 + 3 * t) * PITCH + col0,
                            ap=[[PITCH, 3], [0, F], [1, 1024]],
                        )
                        nc.sync.dma_start(out=xh, in_=srcap)
                        xhs.append(xh)
                    # A-tiles (square trick, ScalarE evac)
                    for t in range(NA2):
                        u = u_ps.tile([128, 1024], FP32, tag="u")
                        for hh in range(2):
                            nc.tensor.matmul(
                                u[:, hh * 512:(hh + 1) * 512],
                                lhsT=sel2s[:, t, :],
                                rhs=xj_ap[:, col0 + hh * 512:
                                          col0 + (hh + 1) * 512],
                                start=True, stop=True,
                            )
                        z = z_sb.tile([128, 1024], BF16, tag="z")
                        nc.scalar.activation(
                            out=z, in_=u,
                            func=mybir.ActivationFunctionType.Square,
                            bias=0.0, scale=float(1.0 / np.sqrt(2.0)),
                        )
                        for i in range(2):
                            nc.tensor.matmul(
                                y1[i * 64:(i + 1) * 64, :],
                                lhsT=was[:, t, (dbase + i) * 64:
                                         (dbase + i + 1) * 64],
                                rhs=z[:, i * 512:(i + 1) * 512],
                                start=(t == 0), stop=False,
                                skip_group_check=True,
                            )
                    # C-tiles: VectorE mul + matmuls
                    for t in range(NC2):
                        z = z_sb.tile([120, 1024], BF16, tag="z")
                        nc.vector.tensor_mul(
                            z, xhs[t], xfr[0:120, col0:col0 + 1024]
                        )
                        for i in range(2):
                            nc.tensor.matmul(
                                y1[i * 64:(i + 1) * 64, :],
                                lhsT=wcs[:, t, (dbase + i) * 64:
                                         (dbase + i + 1) * 64],
                                rhs=z[:, i * 512:(i + 1) * 512],
                                start=False, stop=False,
                                skip_group_check=True,
                            )
                    # correction: s = [x^2 | garbage | x1^2], one mm per d
                    nc.scalar.activation(
                        out=s[0:JX1, :], in_=xj_ap[0:JX1, col0:col0 + 1024],
                        func=mybir.ActivationFunctionType.Square,
                        bias=0.0, scale=1.0,
                    )
                    nc.scalar.activation(
                        out=s[JX1:128, :],
                        in_=xj_ap[JX1:128, col0:col0 + 1024],
                        func=mybir.ActivationFunctionType.Square,
                        bias=0.0, scale=1.0,
                    )
                    for i in range(2):
                        nc.tensor.matmul(
                            y1[i * 64:(i + 1) * 64, :],
                            lhsT=cor2s[:, (dbase + i) * 64:
                                       (dbase + i + 1) * 64],
                            rhs=s[:, i * 512:(i + 1) * 512],
                            start=False, stop=True,
                            skip_group_check=True,
                        )
                    for i in range(2):
                        x2 = x2_sb.tile([O1, 512], BF16, tag="x2")
                        nc.scalar.activation(
                            out=x2, in_=y1[i * 64:(i + 1) * 64, :],
                            func=mybir.ActivationFunctionType.Relu,
                            bias=b1s, scale=1.0,
                        )
                        nc.vector.tensor_add(
                            acc12[0:64, bcol:bcol + 512],
                            acc12[0:64, bcol:bcol + 512], x2
                        )

            # ---- epilogue: transpose (128, B) acc -> (B, 128) fp32 ----
            for bh in range(B // 128):
                outT = o_sb.tile([128, O0 + O1], FP32, tag="outT")
                pt = y_ps.tile([128, 128], FP32, tag="y")
                nc.tensor.transpose(
                    pt, acc12[:, bh * 128:(bh + 1) * 128], ident
                )
                nc.vector.tensor_copy(out=outT[:, 0:64], in_=pt[:, 64:128])
                nc.vector.tensor_copy(out=outT[:, 64:128], in_=pt[:, 0:64])
                nc.sync.dma_start(
                    out=out[bh * 128:(bh + 1) * 128, :], in_=outT
                )

    nc.compile()
    return nc


_NC_CACHE = {}
LAST_RESULT = None


def _get_nc(reps=1):
    if reps not in _NC_CACHE:
        _NC_CACHE[reps] = _build_bass(reps)
    return _NC_CACHE[reps]


def _host_prep(x, W0, b0, W1, b1):
    """Per-core input maps. Core c handles d-channels [4c, 4c+4)."""
    x = np.asarray(x, dtype=np.float32)
    W0 = np.asarray(W0, dtype=np.float32)
    W1 = np.asarray(W1, dtype=np.float32)

    # fold W0 to upper-triangular pairs: (o, 820, d)
    W0r = W0.reshape(O0, F, F, D)
    iu0, iu1 = np.triu_indices(F)
    W0f = W0r[:, iu0, iu1, :] + np.where(
        (iu0 != iu1)[None, :, None], W0r[:, iu1, iu0, :], 0.0
    )
    W1r = W1.reshape(O1, O0, F, D)

    in_maps = []
    for c in range(NCORES):
        ds = slice(4 * c, 4 * c + 4)
        xc = x[:, :, ds]                           # (2048, 40, 4)

        # col layout: pair p, chunk k, d_par i, b_off
        xt = np.empty((F, NPAIR, NCHUNK, 2, BC), dtype=np.float32)
        for p in range(NPAIR):
            for i in range(2):
                d = 2 * p + i
                xt[:, p, :, i, :] = xc[:, :, d].T.reshape(F, NCHUNK, BC)
        xtc = np.ascontiguousarray(xt.reshape(F, PITCH)).astype(NPBF16)

        # host z0: (128, ND1 * PITCH); tile t rows = folded pairs
        z0h = np.zeros((128, ND1, NPAIR, NCHUNK, 2, BC), dtype=NPBF16)
        prod = xc[:, iu0, :] * xc[:, iu1, :]       # (B, 820, 4) fp32
        for t in range(ND1):
            n = min(128, 820 - t * 128)
            blk = prod[:, t * 128:t * 128 + n, :]  # (B, n, 4)
            # -> (n, d, B) -> (n, pair, i, chunk, b) -> (n, pair, chunk, i, b)
            pb = blk.transpose(1, 2, 0).reshape(n, NPAIR, 2, NCHUNK, BC)
            z0h[:n, t] = pb.transpose(0, 1, 3, 2, 4).astype(NPBF16)
        z0hc = np.ascontiguousarray(z0h.reshape(128, ND1 * PITCH))

        # layer-1 weights (128, ND1, DC*64) ordered [tile][d][o]
        w1w = np.zeros((128, ND1, DC, O0), dtype=np.float32)
        w0fc = W0f[:, :, ds]                       # (64, 820, 4)
        for t in range(ND1):
            n = min(128, 820 - t * 128)
            w1w[:n, t] = w0fc[:, t * 128:t * 128 + n, :].transpose(1, 2, 0)
        w1tc = np.ascontiguousarray(
            w1w.reshape(128, ND1 * DC * O0)).astype(NPBF16)

        # layer-2 A-tile weights
        w1c = W1r[:, :, :, ds]                     # (64, 64, 40, 4)
        wa = np.zeros((128, NA2, DC, O0), dtype=np.float32)
        for t in range(NA2):
            rows = L2_PAIRS[t * 128:(t + 1) * 128]
            for r, (h, f) in enumerate(rows):
                wa[r, t] = w1c[:, h, f, :].T
        wac = np.ascontiguousarray(
            wa.reshape(128, NA2 * DC * O0)).astype(NPBF16)

        # C-tile weights (120, NC2, DC*64): rows h=3t+p//40, f=p%40
        wcw = np.zeros((120, NC2, DC, O1), dtype=np.float32)
        for t in range(NC2):
            for p in range(120):
                h, f = 3 * t + p // F, p % F
                wcw[p, t] = w1c[:, h, f, :].T
        wcc = np.ascontiguousarray(
            wcw.reshape(120, NC2 * DC * O1)).astype(NPBF16)

        # layer-2 correction
        cor2 = np.zeros((128, DC, O1), dtype=np.float32)
        for (h, f) in L2_PAIRS:
            w = w1c[:, h, f, :]                    # (o, d)
            cor2[f] -= 0.5 * w.T
            cor2[64 + h] -= 0.5 * w.T

        in_maps.append({
            "xt": xtc,
            "z0": z0hc,
            "w1t": w1tc,
            "wa": wac,
            "wc": wcc,
            "sel2": _SEL2,
            "cor2": np.ascontiguousarray(
                cor2.reshape(128, DC * O1)).astype(NPBF16),
            "b0": np.asarray(b0, np.float32).reshape(O0, 1),
            "b1": np.asarray(b1, np.float32).reshape(O1, 1),
        })
    return in_maps


def _build_sels():
    s2 = np.zeros((128, NA2, 128), dtype=NPBF16)
    for t in range(NA2):
        rows = L2_PAIRS[t * 128:(t + 1) * 128]
        for p, (h, f) in enumerate(rows):
            s2[f, t, p] += 1.0
            s2[64 + h, t, p] += 1.0
    return np.ascontiguousarray(s2.reshape(128, NA2 * 128))


_SEL2 = _build_sels()


def kernel(x, W0, b0, W1, b1):
    global LAST_RESULT
    x = np.asarray(x, dtype=np.float32)

    nc = _get_nc()
    in_maps = _host_prep(x, W0, b0, W1, b1)
    res = run_bass_kernel_spmd(nc, in_maps, core_ids=list(range(NCORES)))
    LAST_RESULT = res

    out = np.empty((B, F + O0 + O1), dtype=np.float32)
    out[:, :F] = x.sum(axis=-1)
    acc = np.zeros((B, O0 + O1), dtype=np.float32)
    for c in range(NCORES):
        acc += np.asarray(res.results[c]["out"])
    out[:, F:] = acc
    return out


# revision 21
# speedup vs baseline: 1.3968x; 1.2774x over previous
"""Trainium2 Bass kernel for the 2-layer CIN — v3 (host-z0 + square trick).

Reference computation (per batch b, channel d):
  z0[hf]  = x[h,d,b] * x[f,d,b]            (h,f in 0..39)
  x1[o]   = relu(sum_hf W0[o,hf,d] z0[hf] + b0[o])
  z1[hf]  = x1[h,d,b] * x[f,d,b]           (h in 0..63)
  x2[o]   = relu(sum_hf W1[o,hf,d] z1[hf] + b1[o])
  out[b]  = [sum_d x | sum_d x1 | sum_d x2]            (2048, 168)

Sharding: pure 8-way split of the embedding dim D=32 -> DC=4 channels per
core, full batch per core; host adds the 8 partial d-sums. Makes per-core
weight traffic 8x smaller than a batch split.

Device algorithm (bf16 compute, fp32 PSUM):
  Column space: 8192 = 2 d-pairs x 4 b-chunks x (d_even 512 | d_odd 512).
  * Layer 1: z0 = x (x) x is static, so the HOST precomputes the 820
    symmetric-folded products and the device just streams z0 tiles from
    DRAM into the accumulating matmuls. No on-device build at all.
  * Layer 2 A-tiles (square trick): a 128-col "sel2" stationary with two
    ones per column computes u[p] = x1[h(p)] + x[f(p)] on the PE into PSUM;
    ScalarE Square evacuates z = (u/sqrt2)^2 = u^2/2, exploiting
    x1_h*x_f = (x1_h+x_f)^2/2 - x1_h^2/2 - x_f^2/2; the s-corrections fold
    into one correction matmul per d against s = [x^2 | x1^2].
  * Layer 2 C-tiles: XH = x1[3t+p//40] via SBUF->SBUF partition-replicating
    DMA; z = XH * XFR on VectorE (2x mode) with XFR[p] = x[p%40] resident.
  Real matmuls: per tile, two 64-col stationaries (one per d of the pair)
  accumulate into disjoint PSUM partition halves. ScalarE applies bias+relu
  into the joint source XJ (x rows 0-39, x1 rows 64-127); VectorE
  accumulates d-sums into a (128, B) fp32 acc (acc2 rows 0-63, acc1 rows
  64-127); a PE-transpose epilogue emits (2048, 128) fp32 per core.
"""

import os
from contextlib import ExitStack

import numpy as np
import ml_dtypes

import concourse.bass as bass
import concourse.bacc as bacc
import concourse.tile as tile
from concourse import mybir
from concourse.bass_utils import run_bass_kernel_spmd
from concourse.masks import make_identity

BF16 = mybir.dt.bfloat16
FP32 = mybir.dt.float32
NPBF16 = ml_dtypes.bfloat16

B, F, D = 2048, 40, 32
O0, O1 = 64, 64
NCORES = 8
DC = D // NCORES            # 4 embedding channels per core
NPAIR = DC // 2             # 2 d-pairs
NCHUNK = 4                  # batch chunks of 512
BC = B // NCHUNK            # 512 batch cols per chunk
NG = NPAIR * NCHUNK         # 8 col groups of 1024
PITCH = DC * B              # 8192 free cols of the resident tiles
JX1 = 64                    # x1 rows start at partition 64 of XJ

# --- tunables ---
HC2 = 33                    # layer-2 h-values covered by C-tiles (mult of 3)
NC2 = HC2 * F // 120        # 8 C-tiles per group

L1_PAIRS = [(h, f) for h in range(F) for f in range(h, F)]          # 820
L2_PAIRS = [(h, f) for h in range(HC2, O0) for f in range(F)]       # 1600
ND1 = (len(L1_PAIRS) + 127) // 128                                  # 7
NA2 = (len(L2_PAIRS) + 127) // 128                                  # 13


def _build_bass(reps=1):
    nc = bacc.Bacc()
    xt = nc.declare_dram_parameter("xt", [F, PITCH], BF16, isOutput=False)
    z0 = nc.declare_dram_parameter("z0", [128, ND1 * PITCH], BF16, isOutput=False)
    w1t = nc.declare_dram_parameter("w1t", [128, ND1 * DC * O0], BF16, isOutput=False)
    wa = nc.declare_dram_parameter("wa", [128, NA2 * DC * O0], BF16, isOutput=False)
    wc = nc.declare_dram_parameter("wc", [120, NC2 * DC * O1], BF16, isOutput=False)
    sel2 = nc.declare_dram_parameter("sel2", [128, NA2 * 128], BF16, isOutput=False)
    cor2 = nc.declare_dram_parameter("cor2", [128, DC * O1], BF16, isOutput=False)
    b0 = nc.declare_dram_parameter("b0", [O0, 1], FP32, isOutput=False)
    b1 = nc.declare_dram_parameter("b1", [O1, 1], FP32, isOutput=False)
    out = nc.declare_dram_parameter("out", [B, O0 + O1], FP32, isOutput=True)

    with ExitStack() as ctx:
        tc = ctx.enter_context(tile.TileContext(nc))
        singles = ctx.enter_context(tc.tile_pool(name="singles", bufs=1))
        u_ps = ctx.enter_context(tc.tile_pool(name="u_ps", bufs=2, space="PSUM"))
        y_ps = ctx.enter_context(tc.tile_pool(name="y_ps", bufs=4, space="PSUM"))
        z_sb = ctx.enter_context(tc.tile_pool(name="z_sb", bufs=10))
        xh_sb = ctx.enter_context(tc.tile_pool(name="xh_sb", bufs=12))
        x2_sb = ctx.enter_context(tc.tile_pool(name="x2_sb", bufs=4))
        o_sb = ctx.enter_context(tc.tile_pool(name="o_sb", bufs=2))
        s_sb = ctx.enter_context(tc.tile_pool(name="s_sb", bufs=8))
        z0_sb = ctx.enter_context(tc.tile_pool(name="z0_sb", bufs=8))

        # ---- resident tensors ----
        xj = singles.tile([128, PITCH], BF16)   # x rows 0-39, x1 rows 64-127
        xfr = singles.tile([128, PITCH], BF16)  # x[p%40], pad 120-127
        w1s = singles.tile([128, ND1, DC * O0], BF16)
        was = singles.tile([128, NA2, DC * O0], BF16)
        wcs = singles.tile([120, NC2, DC * O1], BF16)
        sel2s = singles.tile([128, NA2, 128], BF16)
        cor2s = singles.tile([128, DC * O1], BF16)
        b0s = singles.tile([O0, 1], FP32)
        b1s = singles.tile([O1, 1], FP32)
        acc12 = singles.tile([128, B], FP32)    # rows 0-63 acc2, 64-127 acc1
        ident = singles.tile([128, 128], FP32)
        make_identity(nc, ident)

        xt_ap = xt[:]
        rep_src = bass.AP(
            tensor=xt_ap.tensor, offset=xt_ap.offset,
            ap=[[0, 3], [PITCH, F], [1, PITCH]],
        )
        pad_src = bass.AP(
            tensor=xt_ap.tensor, offset=xt_ap.offset,
            ap=[[PITCH, 8], [1, PITCH]],
        )
        pad24_src = bass.AP(
            tensor=xt_ap.tensor, offset=xt_ap.offset,
            ap=[[PITCH, JX1 - F], [1, PITCH]],
        )

        def load_inputs():
            nc.gpsimd.dma_start(out=xj[0:F, :], in_=xt[:])
            nc.gpsimd.dma_start(out=xj[F:JX1, :], in_=pad24_src)
            nc.gpsimd.dma_start(out=xfr[0:3 * F, :], in_=rep_src)
            nc.gpsimd.dma_start(out=xfr[3 * F:128, :], in_=pad_src)
            nc.gpsimd.dma_start(out=sel2s, in_=sel2[:])
            nc.gpsimd.dma_start(out=cor2s, in_=cor2[:])
            nc.gpsimd.dma_start(out=b0s, in_=b0[:])
            nc.gpsimd.dma_start(out=b1s, in_=b1[:])
            nc.sync.dma_start(out=w1s, in_=w1t[:])
            nc.sync.dma_start(out=was, in_=wa[:])
            nc.gpsimd.dma_start(out=wcs, in_=wc[:])

        xj_ap = xj[:]

        load_inputs()
        for rep in range(reps):
            nc.vector.memset(acc12, 0.0)
            for pair in range(NPAIR):
                dbase = pair * 2
                # stream this pair's z0 tiles (prefetched via pool bufs)
                z0ts = []
                for t in range(ND1):
                    zt = z0_sb.tile([128, NCHUNK * 1024], BF16, tag="z0")
                    nc.gpsimd.dma_start(
                        out=zt,
                        in_=z0[:, t * PITCH + pair * NCHUNK * 1024:
                               t * PITCH + (pair + 1) * NCHUNK * 1024],
                    )
                    z0ts.append(zt)

                for chunk in range(NCHUNK):
                    g = pair * NCHUNK + chunk
                    col0 = g * 1024
                    ccol = chunk * 1024
                    bcol = chunk * 512
                    s = s_sb.tile([128, 1024], BF16, tag="s")

                    # ---- layer 1: stream host-built z0 into matmuls ----
                    y0 = y_ps.tile([128, 512], FP32, tag="y", name=f"y0_{g}")
                    for t in range(ND1):
                        for i in range(2):
                            nc.tensor.matmul(
                                y0[i * 64:(i + 1) * 64, :],
                                lhsT=w1s[:, t, (dbase + i) * 64:
                                         (dbase + i + 1) * 64],
                                rhs=z0ts[t][:, ccol + i * 512:
                                            ccol + (i + 1) * 512],
                                start=(t == 0), stop=(t == ND1 - 1),
                                skip_group_check=True,
                            )
                    for i in range(2):
                        nc.scalar.activation(
                            out=xj[JX1:JX1 + O0,
                                   col0 + i * 512: col0 + (i + 1) * 512],
                            in_=y0[i * 64:(i + 1) * 64, :],
                            func=mybir.ActivationFunctionType.Relu,
                            bias=b0s, scale=1.0,
                        )
                        nc.vector.tensor_add(
                            acc12[64:128, bcol:bcol + 512],
                            acc12[64:128, bcol:bcol + 512],
                            xj_ap[JX1:JX1 + O0,
                                  col0 + i * 512:col0 + (i + 1) * 512],
                        )

                    # ---- layer 2 ----
                    y1 = y_ps.tile([128, 512], FP32, tag="y", name=f"y1_{g}")
                    # C-tile replication DMAs first (overlap A compute)
                    xhs = []
                    for t in range(NC2):
                        xh = xh_sb.tile([120, 1024], BF16, tag="xh")
                        srcap = bass.AP(
                            tensor=xj_ap.tensor,
                            offset=(JX1 + 3 * t) * PITCH + col0,
                            ap=[[PITCH, 3], [0, F], [1, 1024]],
                        )
                        nc.sync.dma_start(out=xh, in_=srcap)
                        xhs.append(xh)
                    # A-tiles (square trick, ScalarE evac)
                    for t in range(NA2):
                        u = u_ps.tile([128, 1024], FP32, tag="u")
                        for hh in range(2):
                            nc.tensor.matmul(
                                u[:, hh * 512:(hh + 1) * 512],
                                lhsT=sel2s[:, t, :],
                                rhs=xj_ap[:, col0 + hh * 512:
                                          col0 + (hh + 1) * 512],
                                start=True, stop=True,
                            )
                        z = z_sb.tile([128, 1024], BF16, tag="z")
                        nc.scalar.activation(
                            out=z, in_=u,
                            func=mybir.ActivationFunctionType.Square,
                            bias=0.0, scale=float(1.0 / np.sqrt(2.0)),
                        )
                        for i in range(2):
                            nc.tensor.matmul(
                                y1[i * 64:(i + 1) * 64, :],
                                lhsT=was[:, t, (dbase + i) * 64:
                                         (dbase + i + 1) * 64],
                                rhs=z[:, i * 512:(i + 1) * 512],
                                start=(t == 0), stop=False,
                                skip_group_check=True,
                            )
                    # C-tiles: VectorE mul + matmuls
                    for t in range(NC2):
                        z = z_sb.tile([120, 1024], BF16, tag="z")
                        nc.vector.tensor_mul(
                            z, xhs[t], xfr[0:120, col0:col0 + 1024]
                        )
                        for i in range(2):
                            nc.tensor.matmul(
                                y1[i * 64:(i + 1) * 64, :],
                                lhsT=wcs[:, t, (dbase + i) * 64:
                                         (dbase + i + 1) * 64],
                                rhs=z[:, i * 512:(i + 1) * 512],
                                start=False, stop=False,
                                skip_group_check=True,
                            )
                    # correction: s = [x^2 | garbage | x1^2], one mm per d
                    nc.scalar.activation(
                        out=s[0:JX1, :], in_=xj_ap[0:JX1, col0:col0 + 1024],
                        func=mybir.ActivationFunctionType.Square,
                        bias=0.0, scale=1.0,
                    )
                    nc.scalar.activation(
                        out=s[JX1:128, :],
                        in_=xj_ap[JX1:128, col0:col0 + 1024],
                        func=mybir.ActivationFunctionType.Square,
                        bias=0.0, scale=1.0,
                    )
                    for i in range(2):
                        nc.tensor.matmul(
                            y1[i * 64:(i + 1) * 64, :],
                            lhsT=cor2s[:, (dbase + i) * 64:
                                       (dbase + i + 1) * 64],
                            rhs=s[:, i * 512:(i + 1) * 512],
                            start=False, stop=True,
                            skip_group_check=True,
                        )
                    for i in range(2):
                        x2 = x2_sb.tile([O1, 512], BF16, tag="x2")
                        nc.scalar.activation(
                            out=x2, in_=y1[i * 64:(i + 1) * 64, :],
                            func=mybir.ActivationFunctionType.Relu,
                            bias=b1s, scale=1.0,
                        )
                        nc.vector.tensor_add(
                            acc12[0:64, bcol:bcol + 512],
                            acc12[0:64, bcol:bcol + 512], x2
                        )

            # ---- epilogue: transpose (128, B) acc -> (B, 128) fp32 ----
            for bh in range(B // 128):
                outT = o_sb.tile([128, O0 + O1], FP32, tag="outT")
                pt = y_ps.tile([128, 128], FP32, tag="y")
                nc.tensor.transpose(
                    pt, acc12[:, bh * 128:(bh + 1) * 128], ident
                )
                nc.vector.tensor_copy(out=outT[:, 0:64], in_=pt[:, 64:128])
                nc.vector.tensor_copy(out=outT[:, 64:128], in_=pt[:, 0:64])
                nc.sync.dma_start(
                    out=out[bh * 128:(bh + 1) * 128, :], in_=outT
                )

    nc.compile()
    return nc


_NC_CACHE = {}
LAST_RESULT = None


def _get_nc(reps=1):
    if reps not in _NC_CACHE:
        _NC_CACHE[reps] = _build_bass(reps)
    return _NC_CACHE[reps]


def _host_prep(x, W0, b0, W1, b1):
    """Per-core input maps. Core c handles d-channels [4c, 4c+4)."""
    x = np.asarray(x, dtype=np.float32)
    W0 = np.asarray(W0, dtype=np.float32)
    W1 = np.asarray(W1, dtype=np.float32)

    # fold W0 to upper-triangular pairs: (o, 820, d)
    W0r = W0.reshape(O0, F, F, D)
    iu0, iu1 = np.triu_indices(F)
    W0f = W0r[:, iu0, iu1, :] + np.where(
        (iu0 != iu1)[None, :, None], W0r[:, iu1, iu0, :], 0.0
    )
    W1r = W1.reshape(O1, O0, F, D)

    in_maps = []
    for c in range(NCORES):
        ds = slice(4 * c, 4 * c + 4)
        xc = x[:, :, ds]                           # (2048, 40, 4)

        # col layout: pair p, chunk k, d_par i, b_off
        xt = np.empty((F, NPAIR, NCHUNK, 2, BC), dtype=np.float32)
        for p in range(NPAIR):
            for i in range(2):
                d = 2 * p + i
                xt[:, p, :, i, :] = xc[:, :, d].T.reshape(F, NCHUNK, BC)
        xtc = np.ascontiguousarray(xt.reshape(F, PITCH)).astype(NPBF16)

        # host z0: (128, ND1 * PITCH); tile t rows = folded pairs
        z0h = np.zeros((128, ND1, NPAIR, NCHUNK, 2, BC), dtype=NPBF16)
        prod = xc[:, iu0, :] * xc[:, iu1, :]       # (B, 820, 4) fp32
        for t in range(ND1):
            n = min(128, 820 - t * 128)
            blk = prod[:, t * 128:t * 128 + n, :]  # (B, n, 4)
            # -> (n, d, B) -> (n, pair, i, chunk, b) -> (n, pair, chunk, i, b)
            pb = blk.transpose(1, 2, 0).reshape(n, NPAIR, 2, NCHUNK, BC)
            z0h[:n, t] = pb.transpose(0, 1, 3, 2, 4).astype(NPBF16)
        z0hc = np.ascontiguousarray(z0h.reshape(128, ND1 * PITCH))

        # layer-1 weights (128, ND1, DC*64) ordered [tile][d][o]
        w1w = np.zeros((128, ND1, DC, O0), dtype=np.float32)
        w0fc = W0f[:, :, ds]                       # (64, 820, 4)
        for t in range(ND1):
            n = min(128, 820 - t * 128)
            w1w[:n, t] = w0fc[:, t * 128:t * 128 + n, :].transpose(1, 2, 0)
        w1tc = np.ascontiguousarray(
            w1w.reshape(128, ND1 * DC * O0)).astype(NPBF16)

        # layer-2 A-tile weights
        w1c = W1r[:, :, :, ds]                     # (64, 64, 40, 4)
        wa = np.zeros((128, NA2, DC, O0), dtype=np.float32)
        for t in range(NA2):
            rows = L2_PAIRS[t * 128:(t + 1) * 128]
            for r, (h, f) in enumerate(rows):
                wa[r, t] = w1c[:, h, f, :].T
        wac = np.ascontiguousarray(
            wa.reshape(128, NA2 * DC * O0)).astype(NPBF16)

        # C-tile weights (120, NC2, DC*64): rows h=3t+p//40, f=p%40
        wcw = np.zeros((120, NC2, DC, O1), dtype=np.float32)
        for t in range(NC2):
            for p in range(120):
                h, f = 3 * t + p // F, p % F
                wcw[p, t] = w1c[:, h, f, :].T
        wcc = np.ascontiguousarray(
            wcw.reshape(120, NC2 * DC * O1)).astype(NPBF16)

        # layer-2 correction
        cor2 = np.zeros((128, DC, O1), dtype=np.float32)
        for (h, f) in L2_PAIRS:
            w = w1c[:, h, f, :]                    # (o, d)
            cor2[f] -= 0.5 * w.T
            cor2[64 + h] -= 0.5 * w.T

        in_maps.append({
            "xt": xtc,
            "z0": z0hc,
            "w1t": w1tc,
            "wa": wac,
            "wc": wcc,
            "sel2": _SEL2,
            "cor2": np.ascontiguousarray(
                cor2.reshape(128, DC * O1)).astype(NPBF16),
            "b0": np.asarray(b0, np.float32).reshape(O0, 1),
            "b1": np.asarray(b1, np.float32).reshape(O1, 1),
        })
    return in_maps


def _build_sels():
    s2 = np.zeros((128, NA2, 128), dtype=NPBF16)
    for t in range(NA2):
        rows = L2_PAIRS[t * 128:(t + 1) * 128]
        for p, (h, f) in enumerate(rows):
            s2[f, t, p] += 1.0
            s2[64 + h, t, p] += 1.0
    return np.ascontiguousarray(s2.reshape(128, NA2 * 128))


_SEL2 = _build_sels()


def kernel(x, W0, b0, W1, b1):
    global LAST_RESULT
    x = np.asarray(x, dtype=np.float32)

    nc = _get_nc()
    in_maps = _host_prep(x, W0, b0, W1, b1)
    res = run_bass_kernel_spmd(nc, in_maps, core_ids=list(range(NCORES)))
    LAST_RESULT = res

    out = np.empty((B, F + O0 + O1), dtype=np.float32)
    out[:, :F] = x.sum(axis=-1)
    acc = np.zeros((B, O0 + O1), dtype=np.float32)
    for c in range(NCORES):
        acc += np.asarray(res.results[c]["out"])
    out[:, F:] = acc
    return out
